# revision 48
# baseline (speedup 1.0000x reference)
"""Trainium2 Bass kernel for nn_NSMCell (GNN message passing).

Strategy
--------
The reference output is only [N]: a per-graph blend of two segment softmaxes
over per-node scalars.  Both scalars are of the form

    s_i = sum_d w_d * elu( M_g[d, :] @ x_i )

where for "node items" M_g = (sim[g] . W_node_props) * instr[g] and x = node
attr, and for "edge items" M_g = W_edge * instr[g] and x = edge attr.  The
per-graph matrices are built on the host (they are tiny); the device streams
all item columns through the PE + exp/min elu + a weighted partition reduce.
The edge-message scatter (index_add) collapses to a host-side bincount of
per-edge scalars, and the segment softmax + blend run on the host over [N]
values (negligible work).

Sharding/packing: graphs sorted by edge count are striped across the 8
cores, so the 8 graphs sharing a "slot" have similar sizes.  Each slot's
width is the max item count over its 8 cores (rounded to 16); slots are
packed back-to-back (all edge slots, 512-aligned, then all node slots) so
the item stream is dense - no fixed 512 per-graph padding.  One NEFF serves
all cores: slot widths are compile-time constants shared by all 8.

The device walks fixed 512-item regions of the packed stream (d on
partitions, 2 d-chunks side by side in one 2-bank PSUM tile), with matmuls
split at slot (graph) boundaries:
  y[d, e]   = A_g[k, d]^T @ xT[k, e]    edge pieces: fp8e4 DoubleRow
                                        matmuls (K=256 virtualized); node
                                        pieces: bf16 matmuls (node y has
                                        ~4x the variance - fp8 there costs
                                        ~4% logit error)
  E         = exp(y/16)                 ScalarE, PSUM -> SBUF bf16
  EL        = min(E, relu(y)/16 + 1)    one fused custom VectorE op
  s-rows   += (w (x) delta_c)^T @ EL    bf16 matmuls per piece into
                                        separate PSUM col-groups
                                        (concurrent via 32-col array
                                        tiling); kc halves land in
                                        different row halves, host adds
The s accumulator is a 2-bank PSUM tile ([128, 1024]); region R's 512 item
sums land at rows (R%64, 64+R%64), cols (R//64)*512.  Drained in quarters
on ScalarE (overlapped with compute for the first half); the host adds the
kc half-rows and subtracts sum(w) to undo the +1.

The elementwise stage is the wall: DVE reads y from PSUM at 1 elem/lane/cyc
(custom DVE ops have no 2x mode and the PSUM f32 port caps the stream), and
ACT exp is right behind it.  Dense packing minimizes the element count;
fp8 DoubleRow + concurrent s-reduce keep PE under that floor; items stream
as multi-region DMAs (a DMA costs 128 row descriptors regardless of size).

Measured on 8 axon trn2 cores: ~129 us HW exec (baseline 172 us), rel err
1.12e-2 (edge-path fp8).  Engine busy: DVE ~103 us (the floor), ACT ~100,
PE ~98, plus ~13 us NEFF startup + DMA fill and ~5 us drain/teardown.
"""

import numpy as np
import ml_dtypes

BF16 = ml_dtypes.bfloat16
FP8 = ml_dtypes.float8_e4m3
N_CORES = 8
D = 256
TILE = 512  # items per region
A_SCALE = 16.0  # pre-scale on A so fp8e4 quantization stays in normals
USE_FP8 = True  # fp8 DoubleRow matmuls for edge pieces


# ----------------------------------------------------------------------------
# Bass kernel builder (one NEFF shared by all cores)
# ----------------------------------------------------------------------------

_BASS_CACHE = {}


def _get_elup1_op():
    """Register (once) a custom fused DVE op:
    out = min(in0, relu(in1) * s0 + s1).

    With in0 = exp(y/s), in1 = y and s0 = 1/s this computes elu(y/s) + 1 in
    a single VectorE pass."""
    from concourse import dve_ops
    from concourse.dve_spec import Spec, Src0, Src1, C0, C1, relu, minn, \
        lower, _has_src1
    from concourse.dve_uop import DveOpSpec

    name = "ELUP1S_ANT"
    for o in dve_ops.OPS:
        if o.name == name:
            return o

    def ref(in0, in1, s0, s1, imm2):
        return np.minimum(
            in0.astype(np.float32),
            np.maximum(in1.astype(np.float32), 0.0) * s0 + s1,
        ).astype(np.float32)

    spec = Spec(body=minn(Src0, relu(Src1) * C0 + C1), reference=ref)
    row = dve_ops._CUSTOM_DVE_ROW_BASE + len(dve_ops.OPS)
    shas = {}
    for ver in ("v3", "v4"):
        uops = lower(spec, ver=ver)
        shas[ver] = DveOpSpec(name=name, opcode=row, uops=uops,
                              rd1_en=_has_src1(spec)).sha(ver)
    op = dve_ops.DveOp(name, spec, subdim=False, uops_sha=shas)
    dve_ops.OPS.append(op)
    dve_ops.CUSTOM_DVE_SPECS[op.name] = op.spec
    dve_ops._SUB_OPCODE_FOR_NAME[op.name] = row
    return op


def _build_bass(regions, n_edge_regions):
    """Build the Tile/Bass program.

    regions: tuple over schedule positions R of (typ, ri, pieces); typ
      0=node 1=edge; ri = index within that type's item blob; pieces =
      tuple of (gl, off, ny, ns): graph slot, start col within the region,
      y-matmul width (zero padding streamed at region tails), and s-matmul
      width (real items only).  Node regions are interleaved among the
      (fp8, cheaper-on-PE) edge regions so the bf16 node matmul bursts
      amortize against DVE slack.
    """
    key = (regions, n_edge_regions, USE_FP8)
    if key in _BASS_CACHE:
        return _BASS_CACHE[key]

    import concourse.mybir as mybir
    import concourse.tile as tile
    from concourse import bacc

    dt = mybir.dt
    G = 16  # graphs per core
    RT = len(regions)
    RE = n_edge_regions
    RN = RT - RE
    assert RT <= 128
    edge_dt = dt.float8e4 if USE_FP8 else dt.bfloat16

    elup1 = _get_elup1_op()
    nc = bacc.Bacc("TRN2", target_bir_lowering=False)
    items_n_d = nc.dram_tensor("items_n", [128, RN, 2, TILE],
                               dt.bfloat16, kind="ExternalInput")
    items_e_d = nc.dram_tensor("items_e", [128, RE, 2, TILE],
                               edge_dt, kind="ExternalInput")
    # mats_*[p, g, dc, kc, m] = A[g][kc*128+p][dc*128+m] (x A_SCALE)
    mats_n_d = nc.dram_tensor("mats_n", [128, G, 2, 2, 128], dt.bfloat16,
                              kind="ExternalInput")
    mats_e_d = nc.dram_tensor("mats_e", [128, G, 2, 2, 128], edge_dt,
                              kind="ExternalInput")
    wtab_d = nc.dram_tensor("wtab", [128, 2 * 2 * 32 * 32], dt.bfloat16,
                            kind="ExternalInput")
    s_d = nc.dram_tensor("s_out", [128, 2 * TILE], dt.float32,
                         kind="ExternalOutput")

    with tile.TileContext(nc) as tc:
        with (
            tc.tile_pool(name="const", bufs=1) as const_pool,
            tc.tile_pool(name="items", bufs=5) as item_pool,
            tc.tile_pool(name="psum_y", bufs=3, space="PSUM") as ypool,
            tc.tile_pool(name="psum_s", bufs=1, space="PSUM") as spool,
            tc.tile_pool(name="elu", bufs=5) as elu_pool,
            tc.tile_pool(name="sout", bufs=1) as sout_pool,
        ):
            # Consts: ACT HWDGE path, one pool tile per chunk so the chunk
            # DMAs carry no same-tile WAW deps (they'd serialize otherwise).
            MCH = 8  # graphs per mats chunk
            mats_n_sbs = [const_pool.tile([128, MCH, 2, 2, 128], dt.bfloat16,
                                          name=f"matnb{i}", tag=f"matn{i}")
                          for i in range(G // MCH)]
            mats_e_sbs = [const_pool.tile([128, MCH, 2, 2, 128], edge_dt,
                                          name=f"mateb{i}", tag=f"mate{i}")
                          for i in range(G // MCH)]
            wtab_sbs = [const_pool.tile([128, 8 * 128], dt.bfloat16,
                                        name=f"wtabb{i}", tag=f"wtab{i}")
                        for i in range(4)]

            def load_mats(which, ch, lo=0, hi=MCH):
                sbs, d = ((mats_n_sbs, mats_n_d) if which == 0 else
                          (mats_e_sbs, mats_e_d))
                nc.scalar.dma_start(sbs[ch][:, lo:hi, :, :, :],
                                    d[:, ch * MCH + lo:ch * MCH + hi, :, :, :])

            def load_wtab(ch):
                sl = slice(ch * 8 * 128, (ch + 1) * 8 * 128)
                nc.scalar.dma_start(wtab_sbs[ch][:], wtab_d[:, sl])

            # s accumulator: 2 PSUM banks.  Region R (kc half h) accumulates
            # its 512 item sums at rows 64*h + (R%64), cols (R//64)*512.
            # memset first so the drain never reads uninitialized PSUM.
            psum_s = spool.tile([128, 2 * TILE], dt.float32)
            nc.vector.memset(psum_s[:], 0)

            # HAM pre-warm: keep PE busy during the DMA preamble so real
            # matmuls start at 2.4 GHz instead of ramping from 1.2 GHz.
            # Warm output parks in an s-accumulator corner no region uses.
            warm_sb = const_pool.tile([128, 64], dt.bfloat16)
            nc.vector.memset(warm_sb[:], 0)
            for _ in range(24):
                nc.tensor.matmul(psum_s[32:64, TILE:TILE + 64],
                                 warm_sb[:, 0:32], warm_sb[:],
                                 start=True, stop=True, skip_group_check=True)

            # Graph 0's matrices load first (small DMAs) so region 0's
            # matmuls aren't gated on megabyte transfers.  Node matrices
            # are only needed once the edge block (region RE) is reached.
            const_sched = {0: lambda: (load_mats(1, 0, 0, 1), load_wtab(0)),
                           1: lambda: (load_mats(1, 0, 1, 8), load_wtab(1)),
                           10: lambda: load_wtab(2),
                           14: lambda: load_wtab(3),
                           18: lambda: (load_mats(1, 1), load_mats(0, 0)),
                           30: lambda: load_mats(0, 1)}

            def w_sl(typ, kc, c):
                # c-major so region R only depends on wtab chunk c // 8
                ch, cc = divmod(c, 8)
                off = ((cc * 2 + typ) * 2 + kc) * 32
                return wtab_sbs[ch][:, off:off + 32]

            s_sb = sout_pool.tile([128, 2 * TILE], dt.float32)
            pending_s = []
            xq = [None, None]
            for R in range(RT):
                typ, ri, pieces = regions[R]
                is_node = typ == 0
                idt = dt.bfloat16 if is_node else edge_dt
                items_d = items_n_d if is_node else items_e_d
                nq = RN if is_node else RE
                # item DMA: several regions per transfer (fat descriptors)
                QQ = 2 if is_node else 4
                if ri % QQ == 0:
                    k = min(QQ, nq - ri)
                    x = item_pool.tile([128, k, 2, TILE], idt,
                                       tag=f"x{typ}{k}")
                    nc.sync.dma_start(x[:, :, :, :],
                                      items_d[:, ri:ri + k, :, :])
                    xq[typ] = x
                x, pos = xq[typ], ri % QQ
                if R in const_sched:
                    const_sched[R]()

                # both d-chunks side by side in one 2-bank PSUM tile
                y = ypool.tile([128, 2 * TILE], dt.float32, tag="y")
                for dc in range(2):
                    for (gl, off, ny, ns) in pieces:
                        ch, g = divmod(gl, MCH)
                        ysl = y[:, dc * TILE + off:dc * TILE + off + ny]
                        if not is_node and USE_FP8:
                            nc.tensor.matmul(
                                ysl, mats_e_sbs[ch][:, g, dc, :, :],
                                x[:, pos, :, off:off + ny],
                                start=True, stop=True,
                                perf_mode=mybir.MatmulPerfMode.DoubleRow)
                        else:
                            sbs = mats_n_sbs if is_node else mats_e_sbs
                            nc.tensor.matmul(ysl, sbs[ch][:, g, dc, 0, :],
                                             x[:, pos, 0, off:off + ny],
                                             start=True, stop=False)
                            nc.tensor.matmul(ysl, sbs[ch][:, g, dc, 1, :],
                                             x[:, pos, 1, off:off + ny],
                                             start=False, stop=True)
                e_t = elu_pool.tile([128, 2 * TILE], dt.bfloat16, tag="e")
                nc.scalar.activation(e_t[:], y[:],
                                     mybir.ActivationFunctionType.Exp,
                                     scale=1.0 / A_SCALE)
                el_t = elu_pool.tile([128, 2 * TILE], dt.bfloat16, tag="el")
                nc.vector._custom_dve(elup1, out=el_t[:], in0=e_t[:],
                                      in1=y[:], s0=1.0 / A_SCALE, s1=1.0)

                # defer this region's s-reduce matmuls by one region so the
                # ACT->DVE chain has a full region of slack before PE needs
                # el_t.  The kc halves go to different 32-col array groups
                # (concurrent) and different PSUM row halves.
                def s_mms(typ=typ, el_t=el_t, R=R, pieces=pieces):
                    cc, rr = divmod(R, 64)
                    g2, c = divmod(rr, 32)
                    for kc in range(2):
                        r0 = 64 * kc + 32 * g2
                        for pi, (gl, off, ny, ns) in enumerate(pieces):
                            if ns == 0:
                                continue
                            nc.tensor.matmul(
                                psum_s[r0:r0 + 32,
                                       cc * TILE + off:cc * TILE + off + ns],
                                w_sl(typ, kc, c),
                                el_t[:, kc * TILE + off:kc * TILE + off + ns],
                                start=(c == 0 and pi == 0),
                                stop=((c == 31 or R == RT - 1)
                                      and pi == len(pieces) - 1),
                                tile_position=(0, r0), skip_group_check=True)
                pending_s.append(s_mms)
                if len(pending_s) > 2:
                    pending_s.pop(0)()

                # the cc=0 column half is complete once region 63's
                # s-matmuls have run (issued at R=64); drain it on ScalarE
                # (which has slack) in quarter-bank pieces overlapped with
                # the remaining regions, so the end-of-kernel tail only
                # covers the cc=1 half
                if RT > 70 and R in (70, 74, 78, 82) and RT > 84:
                    j = (R - 70) // 4
                    half = slice(j * 128, (j + 1) * 128)
                    nc.scalar.copy(out=s_sb[:, half], in_=psum_s[:, half])
                    nc.scalar.dma_start(s_d[:, half], s_sb[:, half])

            for fn in pending_s:
                fn()

            tail = slice(TILE, 2 * TILE) if RT > 84 else slice(0, 2 * TILE)
            nc.scalar.copy(out=s_sb[:, tail], in_=psum_s[:, tail])
            nc.scalar.dma_start(s_d[:, tail], s_sb[:, tail])

    nc.compile()
    _BASS_CACHE[key] = nc
    return nc


# ----------------------------------------------------------------------------
# Host-side wrapper
# ----------------------------------------------------------------------------

def kernel(instruction_batch, distribution, node_prop_similarities,
           relation_similarity, node_attrs, edge_attrs,
           W_node_props, W_edge, w_node_score, w_rel_score,
           node_indices, edge_batch_indices, edge_indices):
    from concourse.bass_utils import run_bass_kernel_spmd

    ib = np.asarray(instruction_batch, dtype=np.float32)
    dist = np.asarray(distribution, dtype=np.float32)
    sim = np.asarray(node_prop_similarities, dtype=np.float32)
    rsim = np.asarray(relation_similarity, dtype=np.float32)
    na = np.asarray(node_attrs, dtype=np.float32)
    ea = np.asarray(edge_attrs, dtype=np.float32)
    Wp = np.asarray(W_node_props, dtype=np.float32)
    We = np.asarray(W_edge, dtype=np.float32)
    wn = np.asarray(w_node_score, dtype=np.float32)
    wr = np.asarray(w_rel_score, dtype=np.float32)
    ni = np.asarray(node_indices).astype(np.int64)
    ebi = np.asarray(edge_batch_indices).astype(np.int64)
    ei = np.asarray(edge_indices).astype(np.int64)
    src, dst = ei[0], ei[1]

    edge_np_dt = FP8 if USE_FP8 else BF16
    B = ib.shape[0]
    N = na.shape[0]
    G = B // N_CORES  # graphs per core

    cn = np.bincount(ni, minlength=B)
    ce = np.bincount(ebi, minlength=B)
    nstart = np.concatenate([[0], np.cumsum(cn)])
    eperm = np.argsort(ebi, kind="stable")
    estart = np.concatenate([[0], np.cumsum(ce)])

    # Assign graphs to (core, slot): graphs sorted by edge count striped
    # across cores, so the 8 graphs sharing a slot have similar edge counts
    # and the shared slot width (max over cores) stays tight.
    asg = np.argsort(ce, kind="stable").reshape(G, N_CORES).T  # [dev, gl]
    cn_a, ce_a = cn[asg], ce[asg]
    assert cn_a.max() <= TILE, "node segment exceeds one tile"

    def rnd16(v):
        return int(min(TILE, max(16, -16 * (-int(v) // 16))))

    # ---- packed slot layout (shared by all cores) ----
    # edge slots for all graphs first (gl-major, then 512-chunks), 512-pad,
    # then node slots.  Widths are the max over the 8 cores of the slot.
    ej = int(-(-ce_a.max() // TILE))  # edge 512-chunks per graph
    slots = []  # (typ, gl, j, start, width)
    cur = 0
    for gl in range(G):
        for j in range(ej):
            w = rnd16(np.clip(ce_a[:, gl] - j * TILE, 0, TILE).max()) \
                if np.clip(ce_a[:, gl] - j * TILE, 0, TILE).max() > 0 else 0
            if w:
                slots.append((1, gl, j, cur, w))
                cur += w
    cur = -(-cur // TILE) * TILE  # align node block to a region boundary
    n_edge_regions = cur // TILE
    for gl in range(G):
        w = rnd16(cn_a[:, gl].max())
        slots.append((0, gl, 0, cur, w))
        cur += w
    total = -(-cur // TILE) * TILE
    n_regions = total // TILE
    assert n_regions <= 128

    # region piece lists (in type-block coords): intersect slots with each
    # 512-col region; extend the last piece of each region to the region
    # end (streams the zero padding so the elementwise ops never read
    # unwritten PSUM)
    by_type = [[], []]  # [typ] -> list of piece tuples, index ri
    for R in range(n_regions):
        lo, hi = R * TILE, (R + 1) * TILE
        typ = 1 if R < n_edge_regions else 0
        pcs = []
        for (t2, gl, j, a, w) in slots:
            if t2 != typ or a + w <= lo or a >= hi:
                continue
            s0, s1 = max(a, lo), min(a + w, hi)
            pcs.append([gl, s0 - lo, s1 - s0, s1 - s0])
        if not pcs:  # pure padding region (can't happen mid-block)
            pcs.append([0, 0, TILE, 0])
        pcs[-1][2] = TILE - pcs[-1][1]  # extend ny to region end
        by_type[typ].append(tuple(tuple(p) for p in pcs))

    # schedule: edge regions first, node regions last (interleaving the
    # PE-heavier bf16 node regions mid-stream measured worse - the 3-deep
    # y pipeline can't absorb the bursts)
    RE, RN = len(by_type[1]), len(by_type[0])
    node_pos = set(range(n_regions - RN, n_regions))
    schedule, cnt = [], [0, 0]
    for R in range(n_regions):
        typ = 0 if R in node_pos else 1
        if cnt[1] >= RE:
            typ = 0
        elif cnt[0] >= RN:
            typ = 1
        schedule.append((typ, cnt[typ]))
        cnt[typ] += 1
    Rpos = [[None] * RN, [None] * RE]
    regions = []
    for R, (typ, ri) in enumerate(schedule):
        Rpos[typ][ri] = R
        regions.append((typ, ri, by_type[typ][ri]))
    regions = tuple(regions)

    # ---- item columns, transposed + narrowed, packed ----
    na_c = na.astype(BF16)
    ea_c = ea[eperm].astype(edge_np_dt)
    items_n = np.zeros((N_CORES, 128, n_regions - n_edge_regions, 2, TILE),
                      dtype=BF16)
    items_e = np.zeros((N_CORES, 128, n_edge_regions, 2, TILE),
                       dtype=edge_np_dt)

    def put(arr, dev, col0, block):
        # block: [n_items, 256] -> scatter columns col0..col0+n (packed
        # col indices within the type block)
        n = block.shape[0]
        if n == 0:
            return
        bT = block.T.reshape(2, 128, n)  # [kc, p, n]
        j = np.arange(col0, col0 + n)
        tt, jj = j // TILE, j % TILE
        arr[dev][:, tt, 0, jj] = bT[0]
        arr[dev][:, tt, 1, jj] = bT[1]

    nb0 = n_edge_regions * TILE  # node block global offset
    for (typ, gl, j, a, w) in slots:
        for dev in range(N_CORES):
            g = asg[dev, gl]
            if typ == 1:
                c = int(np.clip(ce[g] - j * TILE, 0, TILE))
                blk = ea_c[estart[g] + j * TILE: estart[g] + j * TILE + c]
                put(items_e, dev, a, blk)
            else:
                put(items_n, dev, a - nb0, na_c[nstart[g]:nstart[g + 1]])

    # ---- per-graph matrices A[k, d] (instr folded in), x A_SCALE ----
    C = np.einsum("gp,pde->gde", sim, Wp)
    A_node = (C * ib[:, :, None]).transpose(0, 2, 1) * A_SCALE  # [g, k, d]
    A_edge = (We[None, :, :] * ib[:, :, None]).transpose(0, 2, 1) * A_SCALE

    def mats_blob(A, np_dt):
        # A: [B, 256 k, 256 d] -> [dev, p, g, dc, kc, m], graph-assigned
        Ad = A[asg.reshape(-1)].astype(np_dt).reshape(
            N_CORES, G, 2, 128, 2, 128)
        return np.ascontiguousarray(Ad.transpose(0, 3, 1, 4, 2, 5))

    mats_n = mats_blob(A_node, BF16)
    mats_e = mats_blob(A_edge, edge_np_dt)

    # ---- w tables: wtab[k, ((c*2+typ)*2+kc)*32+m] = w_typ[kc*128+k]*(m==c)
    wt = np.stack([wn, wr]).astype(np.float32)                  # [2, 256]
    eye = np.eye(32, dtype=np.float32)
    wtab = np.einsum("tk,cm->kctm", wt.reshape(2, 2, 128).reshape(4, 128), eye)
    wtab = np.ascontiguousarray(wtab.reshape(128, 32, 2, 2, 32)
                                ).reshape(128, 4 * 32 * 32).astype(BF16)

    # ---- run on 8 cores ----
    nc = _build_bass(regions, n_edge_regions)
    in_maps = [{"items_n": items_n[d], "items_e": items_e[d],
                "mats_n": mats_n[d], "mats_e": mats_e[d], "wtab": wtab}
               for d in range(N_CORES)]
    res = run_bass_kernel_spmd(nc, in_maps, core_ids=list(range(N_CORES)))
    s_rows = np.stack([r["s_out"] for r in res.results])        # [8, 128, 1024]

    # ---- unshard + finish on host ----
    # region R's 512 sums: rows (R%64) + (64 + R%64), cols (R//64)*512
    sum_wn = float(wt[0].astype(BF16).astype(np.float32).sum())
    sum_wr = float(wt[1].astype(BF16).astype(np.float32).sum())
    s_flat = np.empty((N_CORES, n_regions * TILE), np.float32)
    for dev in range(N_CORES):
        S = s_rows[dev]
        for cc in range((n_regions + 63) // 64):
            nt = min(64, n_regions - cc * 64)
            blk = (S[0:nt, cc * TILE:(cc + 1) * TILE]
                   + S[64:64 + nt, cc * TILE:(cc + 1) * TILE])
            s_flat[dev, cc * 64 * TILE:(cc * 64 + nt) * TILE] = blk.reshape(-1)

    def sched_cols(typ, a, n):
        # type-block col range [a, a+n) -> schedule-order s_flat columns
        # (never crosses a region boundary: slots are region-aligned or
        # split below at region granularity)
        c = np.arange(a, a + n)
        base = a - (a % TILE) if typ == 1 else a - nb0 - ((a - nb0) % TILE)
        ri = (c - (nb0 if typ == 0 else 0)) // TILE
        return np.array(Rpos[typ])[ri] * TILE + c % TILE

    state_logits = np.empty(N, np.float32)
    s_e = np.empty(ei.shape[1], np.float32)
    for (typ, gl, j, a, w) in slots:
        for dev in range(N_CORES):
            g = asg[dev, gl]
            if typ == 1:
                c = int(np.clip(ce[g] - j * TILE, 0, TILE))
                if c:
                    s_e[estart[g] + j * TILE: estart[g] + j * TILE + c] = \
                        s_flat[dev, sched_cols(1, a, c)] - sum_wr
            else:
                state_logits[nstart[g]:nstart[g + 1]] = \
                    s_flat[dev, sched_cols(0, a, int(cn[g]))] - sum_wn

    rel_logits = np.bincount(dst[eperm], weights=dist[src[eperm]] * s_e,
                             minlength=N).astype(np.float32)

    def seg_softmax(x):
        mx = np.maximum.reduceat(x, nstart[:-1])
        ex = np.exp(x - mx[ni])
        sm = np.add.reduceat(ex, nstart[:-1])
        return ex / sm[ni]

    r = rsim[ni]
    out = r * seg_softmax(rel_logits) + (1.0 - r) * seg_softmax(state_logits)
    return out.astype(np.float32)


# revision 51
# speedup vs baseline: 1.0163x; 1.0163x over previous
"""Trainium2 Bass kernel for nn_NSMCell (GNN message passing).

Strategy
--------
The reference output is only [N]: a per-graph blend of two segment softmaxes
over per-node scalars.  Both scalars are of the form

    s_i = sum_d w_d * elu( M_g[d, :] @ x_i )

where for "node items" M_g = (sim[g] . W_node_props) * instr[g] and x = node
attr, and for "edge items" M_g = W_edge * instr[g] and x = edge attr.  The
per-graph matrices are built on the host (they are tiny); the device streams
all item columns through the PE + exp/min elu + a weighted partition reduce.
The edge-message scatter (index_add) collapses to a host-side bincount of
per-edge scalars, and the segment softmax + blend run on the host over [N]
values (negligible work).

Sharding/packing: graphs sorted by edge count are striped across the 8
cores, so the 8 graphs sharing a "slot" have similar sizes.  Each slot's
width is the max item count over its 8 cores (rounded to 16); slots are
packed back-to-back (all edge slots, 512-aligned, then all node slots) so
the item stream is dense - no fixed 512 per-graph padding.  One NEFF serves
all cores: slot widths are compile-time constants shared by all 8.

The device walks fixed 512-item regions of the packed stream (d on
partitions, 2 d-chunks side by side in one 2-bank PSUM tile), with matmuls
split at slot (graph) boundaries:
  y[d, e]   = A_g[k, d]^T @ xT[k, e]    edge pieces: fp8e4 DoubleRow
                                        matmuls (K=256 virtualized); node
                                        pieces: bf16 matmuls (node y has
                                        ~4x the variance - fp8 there costs
                                        ~4% logit error)
  E         = exp(y/16)                 ScalarE, PSUM -> SBUF bf16
  EL        = min(E, relu(y)/16 + 1)    one fused custom VectorE op
  s-rows   += (w (x) delta_c)^T @ EL    bf16 matmuls per piece into
                                        separate PSUM col-groups
                                        (concurrent via 32-col array
                                        tiling); kc halves land in
                                        different row halves, host adds
The s accumulator is a 2-bank PSUM tile ([128, 1024]); region R's 512 item
sums land at rows (R%64, 64+R%64), cols (R//64)*512.  Drained in quarters
on ScalarE (overlapped with compute for the first half); the host adds the
kc half-rows and subtracts sum(w) to undo the +1.

The elementwise stage is the wall: DVE reads y from PSUM at 1 elem/lane/cyc
(custom DVE ops have no 2x mode and the PSUM f32 port caps the stream), and
ACT exp is right behind it.  Dense packing minimizes the element count;
fp8 DoubleRow + concurrent s-reduce keep PE under that floor; items stream
as multi-region DMAs (a DMA costs 128 row descriptors regardless of size).

Measured on 8 axon trn2 cores: ~129 us HW exec (baseline 172 us), rel err
1.12e-2 (edge-path fp8).  Engine busy: DVE ~103 us (the floor), ACT ~100,
PE ~98, plus ~13 us NEFF startup + DMA fill and ~5 us drain/teardown.
"""

import numpy as np
import ml_dtypes

BF16 = ml_dtypes.bfloat16
FP8 = ml_dtypes.float8_e4m3
N_CORES = 8
D = 256
TILE = 512  # items per region
A_SCALE = 16.0  # pre-scale on A so fp8e4 quantization stays in normals
USE_FP8 = True  # fp8 DoubleRow matmuls for edge pieces


# ----------------------------------------------------------------------------
# Bass kernel builder (one NEFF shared by all cores)
# ----------------------------------------------------------------------------

_BASS_CACHE = {}


def _get_elup1_op():
    """Register (once) a custom fused DVE op:
    out = min(in0, relu(in1) * s0 + s1).

    With in0 = exp(y/s), in1 = y and s0 = 1/s this computes elu(y/s) + 1 in
    a single VectorE pass."""
    from concourse import dve_ops
    from concourse.dve_spec import Spec, Src0, Src1, C0, C1, relu, minn, \
        lower, _has_src1
    from concourse.dve_uop import DveOpSpec

    name = "ELUP1S_ANT"
    for o in dve_ops.OPS:
        if o.name == name:
            return o

    def ref(in0, in1, s0, s1, imm2):
        return np.minimum(
            in0.astype(np.float32),
            np.maximum(in1.astype(np.float32), 0.0) * s0 + s1,
        ).astype(np.float32)

    spec = Spec(body=minn(Src0, relu(Src1) * C0 + C1), reference=ref)
    row = dve_ops._CUSTOM_DVE_ROW_BASE + len(dve_ops.OPS)
    shas = {}
    for ver in ("v3", "v4"):
        uops = lower(spec, ver=ver)
        shas[ver] = DveOpSpec(name=name, opcode=row, uops=uops,
                              rd1_en=_has_src1(spec)).sha(ver)
    op = dve_ops.DveOp(name, spec, subdim=False, uops_sha=shas)
    dve_ops.OPS.append(op)
    dve_ops.CUSTOM_DVE_SPECS[op.name] = op.spec
    dve_ops._SUB_OPCODE_FOR_NAME[op.name] = row
    return op


def _build_bass(regions, n_edge_regions):
    """Build the Tile/Bass program.

    regions: tuple over schedule positions R of (typ, ri, pieces); typ
      0=node 1=edge; ri = index within that type's item blob; pieces =
      tuple of (gl, off, ny, ns): graph slot, start col within the region,
      y-matmul width (zero padding streamed at region tails), and s-matmul
      width (real items only).  Node regions are interleaved among the
      (fp8, cheaper-on-PE) edge regions so the bf16 node matmul bursts
      amortize against DVE slack.
    """
    key = (regions, n_edge_regions, USE_FP8)
    if key in _BASS_CACHE:
        return _BASS_CACHE[key]

    import concourse.mybir as mybir
    import concourse.tile as tile
    from concourse import bacc

    dt = mybir.dt
    G = 16  # graphs per core
    RT = len(regions)
    RE = n_edge_regions
    RN = RT - RE
    assert RT <= 128
    edge_dt = dt.float8e4 if USE_FP8 else dt.bfloat16

    elup1 = _get_elup1_op()
    nc = bacc.Bacc("TRN2", target_bir_lowering=False)
    items_n_d = nc.dram_tensor("items_n", [128, RN, 2, TILE],
                               dt.bfloat16, kind="ExternalInput")
    items_e_d = nc.dram_tensor("items_e", [128, RE, 2, TILE],
                               edge_dt, kind="ExternalInput")
    # mats_*[p, g, dc, kc, m] = A[g][kc*128+p][dc*128+m] (x A_SCALE)
    mats_n_d = nc.dram_tensor("mats_n", [128, G, 2, 2, 128], dt.bfloat16,
                              kind="ExternalInput")
    mats_e_d = nc.dram_tensor("mats_e", [128, G, 2, 2, 128], edge_dt,
                              kind="ExternalInput")
    wtab_d = nc.dram_tensor("wtab", [128, 2 * 2 * 32 * 32], dt.bfloat16,
                            kind="ExternalInput")
    s_d = nc.dram_tensor("s_out", [128, 2 * TILE], dt.float32,
                         kind="ExternalOutput")

    with tile.TileContext(nc) as tc:
        with (
            tc.tile_pool(name="const", bufs=1) as const_pool,
            tc.tile_pool(name="items", bufs=5) as item_pool,
            tc.tile_pool(name="psum_y", bufs=3, space="PSUM") as ypool,
            tc.tile_pool(name="psum_s", bufs=1, space="PSUM") as spool,
            tc.tile_pool(name="elu", bufs=4) as elu_pool,
            tc.tile_pool(name="sout", bufs=1) as sout_pool,
        ):
            # Consts: ACT HWDGE path, one pool tile per chunk so the chunk
            # DMAs carry no same-tile WAW deps (they'd serialize otherwise).
            MCH = 8  # graphs per mats chunk
            mats_n_sbs = [const_pool.tile([128, MCH, 2, 2, 128], dt.bfloat16,
                                          name=f"matnb{i}", tag=f"matn{i}")
                          for i in range(G // MCH)]
            mats_e_sbs = [const_pool.tile([128, MCH, 2, 2, 128], edge_dt,
                                          name=f"mateb{i}", tag=f"mate{i}")
                          for i in range(G // MCH)]
            wtab_sbs = [const_pool.tile([128, 8 * 128], dt.bfloat16,
                                        name=f"wtabb{i}", tag=f"wtab{i}")
                        for i in range(4)]

            def load_mats(which, ch, lo=0, hi=MCH):
                sbs, d = ((mats_n_sbs, mats_n_d) if which == 0 else
                          (mats_e_sbs, mats_e_d))
                nc.scalar.dma_start(sbs[ch][:, lo:hi, :, :, :],
                                    d[:, ch * MCH + lo:ch * MCH + hi, :, :, :])

            def load_wtab(ch):
                sl = slice(ch * 8 * 128, (ch + 1) * 8 * 128)
                nc.scalar.dma_start(wtab_sbs[ch][:], wtab_d[:, sl])

            # s accumulator: 2 PSUM banks.  Region R (kc half h) accumulates
            # its 512 item sums at rows 64*h + (R%64), cols (R//64)*512.
            # memset first so the drain never reads uninitialized PSUM.
            psum_s = spool.tile([128, 2 * TILE], dt.float32)
            nc.vector.memset(psum_s[:], 0)

            # HAM pre-warm: keep PE busy during the DMA preamble so real
            # matmuls start at 2.4 GHz instead of ramping from 1.2 GHz.
            # Warm output parks in an s-accumulator corner no region uses.
            warm_sb = const_pool.tile([128, 64], dt.bfloat16)
            nc.vector.memset(warm_sb[:], 0)
            for _ in range(24):
                nc.tensor.matmul(psum_s[32:64, TILE:TILE + 64],
                                 warm_sb[:, 0:32], warm_sb[:],
                                 start=True, stop=True, skip_group_check=True)

            # Graph 0's matrices load first (small DMAs) so region 0's
            # matmuls aren't gated on megabyte transfers.  Node matrices
            # are only needed once the edge block (region RE) is reached.
            const_sched = {0: lambda: (load_mats(1, 0, 0, 1), load_wtab(0)),
                           1: lambda: (load_mats(1, 0, 1, 8), load_wtab(1)),
                           3: lambda: load_wtab(2),
                           5: lambda: load_wtab(3),
                           8: lambda: (load_mats(1, 1), load_mats(0, 0)),
                           20: lambda: load_mats(0, 1)}

            def w_sl(typ, kc, c):
                # c-major so region R only depends on wtab chunk c // 8
                ch, cc = divmod(c, 8)
                off = ((cc * 2 + typ) * 2 + kc) * 32
                return wtab_sbs[ch][:, off:off + 32]

            s_sb = sout_pool.tile([128, 2 * TILE], dt.float32)
            pending_s = []
            xq = [None, None]
            for R in range(RT):
                typ, ri, pieces = regions[R]
                is_node = typ == 0
                idt = dt.bfloat16 if is_node else edge_dt
                items_d = items_n_d if is_node else items_e_d
                nq = RN if is_node else RE
                # item DMA: several regions per transfer (fat descriptors)
                QQ = 2 if is_node else 4
                if ri % QQ == 0:
                    k = min(QQ, nq - ri)
                    x = item_pool.tile([128, k, 2, TILE], idt,
                                       tag=f"x{typ}{k}")
                    nc.sync.dma_start(x[:, :, :, :],
                                      items_d[:, ri:ri + k, :, :])
                    xq[typ] = x
                x, pos = xq[typ], ri % QQ
                if R in const_sched:
                    const_sched[R]()

                # both d-chunks side by side in one 2-bank PSUM tile
                y = ypool.tile([128, 2 * TILE], dt.float32, tag="y")
                for dc in range(2):
                    for (gl, off, ny, ns) in pieces:
                        ch, g = divmod(gl, MCH)
                        ysl = y[:, dc * TILE + off:dc * TILE + off + ny]
                        if not is_node and USE_FP8:
                            nc.tensor.matmul(
                                ysl, mats_e_sbs[ch][:, g, dc, :, :],
                                x[:, pos, :, off:off + ny],
                                start=True, stop=True,
                                perf_mode=mybir.MatmulPerfMode.DoubleRow)
                        else:
                            sbs = mats_n_sbs if is_node else mats_e_sbs
                            nc.tensor.matmul(ysl, sbs[ch][:, g, dc, 0, :],
                                             x[:, pos, 0, off:off + ny],
                                             start=True, stop=False)
                            nc.tensor.matmul(ysl, sbs[ch][:, g, dc, 1, :],
                                             x[:, pos, 1, off:off + ny],
                                             start=False, stop=True)
                e_t = elu_pool.tile([128, 2 * TILE], dt.bfloat16, tag="e")
                nc.scalar.activation(e_t[:], y[:],
                                     mybir.ActivationFunctionType.Exp,
                                     scale=1.0 / A_SCALE)
                el_t = elu_pool.tile([128, 2 * TILE], dt.bfloat16, tag="el")
                nc.vector._custom_dve(elup1, out=el_t[:], in0=e_t[:],
                                      in1=y[:], s0=1.0 / A_SCALE, s1=1.0)

                # defer this region's s-reduce matmuls by one region so the
                # ACT->DVE chain has a full region of slack before PE needs
                # el_t.  The kc halves go to different 32-col array groups
                # (concurrent) and different PSUM row halves.
                def s_mms(typ=typ, el_t=el_t, R=R, pieces=pieces):
                    cc, rr = divmod(R, 64)
                    g2, c = divmod(rr, 32)
                    for kc in range(2):
                        r0 = 64 * kc + 32 * g2
                        for pi, (gl, off, ny, ns) in enumerate(pieces):
                            if ns == 0:
                                continue
                            nc.tensor.matmul(
                                psum_s[r0:r0 + 32,
                                       cc * TILE + off:cc * TILE + off + ns],
                                w_sl(typ, kc, c),
                                el_t[:, kc * TILE + off:kc * TILE + off + ns],
                                start=(c == 0 and pi == 0),
                                stop=((c == 31 or R == RT - 1)
                                      and pi == len(pieces) - 1),
                                tile_position=(0, r0), skip_group_check=True)
                pending_s.append(s_mms)
                if len(pending_s) > 1:
                    pending_s.pop(0)()

                # the cc=0 column half is complete once region 63's
                # s-matmuls have run (issued at R=64); drain it on ScalarE
                # (which has slack) in quarter-bank pieces overlapped with
                # the remaining regions, so the end-of-kernel tail only
                # covers the cc=1 half
                if RT > 70 and R in (70, 74, 78, 82) and RT > 84:
                    j = (R - 70) // 4
                    half = slice(j * 128, (j + 1) * 128)
                    nc.scalar.copy(out=s_sb[:, half], in_=psum_s[:, half])
                    nc.scalar.dma_start(s_d[:, half], s_sb[:, half])

            for fn in pending_s:
                fn()

            tail = slice(TILE, 2 * TILE) if RT > 84 else slice(0, 2 * TILE)
            nc.scalar.copy(out=s_sb[:, tail], in_=psum_s[:, tail])
            nc.scalar.dma_start(s_d[:, tail], s_sb[:, tail])

    nc.compile()
    _BASS_CACHE[key] = nc
    return nc


# ----------------------------------------------------------------------------
# Host-side wrapper
# ----------------------------------------------------------------------------

def kernel(instruction_batch, distribution, node_prop_similarities,
           relation_similarity, node_attrs, edge_attrs,
           W_node_props, W_edge, w_node_score, w_rel_score,
           node_indices, edge_batch_indices, edge_indices):
    from concourse.bass_utils import run_bass_kernel_spmd

    ib = np.asarray(instruction_batch, dtype=np.float32)
    dist = np.asarray(distribution, dtype=np.float32)
    sim = np.asarray(node_prop_similarities, dtype=np.float32)
    rsim = np.asarray(relation_similarity, dtype=np.float32)
    na = np.asarray(node_attrs, dtype=np.float32)
    ea = np.asarray(edge_attrs, dtype=np.float32)
    Wp = np.asarray(W_node_props, dtype=np.float32)
    We = np.asarray(W_edge, dtype=np.float32)
    wn = np.asarray(w_node_score, dtype=np.float32)
    wr = np.asarray(w_rel_score, dtype=np.float32)
    ni = np.asarray(node_indices).astype(np.int64)
    ebi = np.asarray(edge_batch_indices).astype(np.int64)
    ei = np.asarray(edge_indices).astype(np.int64)
    src, dst = ei[0], ei[1]

    edge_np_dt = FP8 if USE_FP8 else BF16
    B = ib.shape[0]
    N = na.shape[0]
    G = B // N_CORES  # graphs per core

    cn = np.bincount(ni, minlength=B)
    ce = np.bincount(ebi, minlength=B)
    nstart = np.concatenate([[0], np.cumsum(cn)])
    eperm = np.argsort(ebi, kind="stable")
    estart = np.concatenate([[0], np.cumsum(ce)])

    # Assign graphs to (core, slot), sorted-striped so the 8 graphs
    # sharing a slot have similar sizes and the shared slot width (max
    # over cores) stays tight.  Node and edge items are independent
    # computations until the host softmax, so they get SEPARATE
    # assignments (a graph's nodes need not share a core with its edges):
    # edges striped by edge count, nodes striped by node count.
    asg = np.argsort(ce, kind="stable").reshape(G, N_CORES).T  # [dev, gl]
    asg_n = np.argsort(cn, kind="stable").reshape(G, N_CORES).T
    cn_a, ce_a = cn[asg_n], ce[asg]
    assert cn_a.max() <= TILE, "node segment exceeds one tile"

    def rnd16(v):
        return int(min(TILE, max(16, -8 * (-int(v) // 8))))

    # ---- packed slot layout (shared by all cores) ----
    # edge slots for all graphs first (gl-major, then 512-chunks), 512-pad,
    # then node slots.  Widths are the max over the 8 cores of the slot.
    ej = int(-(-ce_a.max() // TILE))  # edge 512-chunks per graph
    slots = []  # (typ, gl, j, start, width)
    cur = 0
    for gl in range(G):
        for j in range(ej):
            w = rnd16(np.clip(ce_a[:, gl] - j * TILE, 0, TILE).max()) \
                if np.clip(ce_a[:, gl] - j * TILE, 0, TILE).max() > 0 else 0
            if w:
                slots.append((1, gl, j, cur, w))
                cur += w
    cur = -(-cur // TILE) * TILE  # align node block to a region boundary
    n_edge_regions = cur // TILE
    for gl in range(G):
        w = rnd16(cn_a[:, gl].max())
        slots.append((0, gl, 0, cur, w))
        cur += w
    total = -(-cur // TILE) * TILE
    n_regions = total // TILE
    assert n_regions <= 128

    # region piece lists (in type-block coords): intersect slots with each
    # 512-col region; extend the last piece of each region to the region
    # end (streams the zero padding so the elementwise ops never read
    # unwritten PSUM)
    by_type = [[], []]  # [typ] -> list of piece tuples, index ri
    for R in range(n_regions):
        lo, hi = R * TILE, (R + 1) * TILE
        typ = 1 if R < n_edge_regions else 0
        pcs = []
        for (t2, gl, j, a, w) in slots:
            if t2 != typ or a + w <= lo or a >= hi:
                continue
            s0, s1 = max(a, lo), min(a + w, hi)
            pcs.append([gl, s0 - lo, s1 - s0, s1 - s0])
        if not pcs:  # pure padding region (can't happen mid-block)
            pcs.append([0, 0, TILE, 0])
        pcs[-1][2] = TILE - pcs[-1][1]  # extend ny to region end
        by_type[typ].append(tuple(tuple(p) for p in pcs))

    # schedule: edge regions first, node regions last (interleaving the
    # PE-heavier bf16 node regions mid-stream measured worse - the 3-deep
    # y pipeline can't absorb the bursts)
    RE, RN = len(by_type[1]), len(by_type[0])
    node_pos = set(range(n_regions - RN, n_regions))
    schedule, cnt = [], [0, 0]
    for R in range(n_regions):
        typ = 0 if R in node_pos else 1
        if cnt[1] >= RE:
            typ = 0
        elif cnt[0] >= RN:
            typ = 1
        schedule.append((typ, cnt[typ]))
        cnt[typ] += 1
    Rpos = [[None] * RN, [None] * RE]
    regions = []
    for R, (typ, ri) in enumerate(schedule):
        Rpos[typ][ri] = R
        regions.append((typ, ri, by_type[typ][ri]))
    regions = tuple(regions)

    # ---- item columns, transposed + narrowed, packed ----
    na_c = na.astype(BF16)
    ea_c = ea[eperm].astype(edge_np_dt)
    items_n = np.zeros((N_CORES, 128, n_regions - n_edge_regions, 2, TILE),
                      dtype=BF16)
    items_e = np.zeros((N_CORES, 128, n_edge_regions, 2, TILE),
                       dtype=edge_np_dt)

    def put(arr, dev, col0, block):
        # block: [n_items, 256] -> scatter columns col0..col0+n (packed
        # col indices within the type block)
        n = block.shape[0]
        if n == 0:
            return
        bT = block.T.reshape(2, 128, n)  # [kc, p, n]
        j = np.arange(col0, col0 + n)
        tt, jj = j // TILE, j % TILE
        arr[dev][:, tt, 0, jj] = bT[0]
        arr[dev][:, tt, 1, jj] = bT[1]

    nb0 = n_edge_regions * TILE  # node block global offset
    for (typ, gl, j, a, w) in slots:
        for dev in range(N_CORES):
            g = (asg if typ == 1 else asg_n)[dev, gl]
            if typ == 1:
                c = int(np.clip(ce[g] - j * TILE, 0, TILE))
                blk = ea_c[estart[g] + j * TILE: estart[g] + j * TILE + c]
                put(items_e, dev, a, blk)
            else:
                put(items_n, dev, a - nb0, na_c[nstart[g]:nstart[g + 1]])

    # ---- per-graph matrices A[k, d] (instr folded in), x A_SCALE ----
    C = np.einsum("gp,pde->gde", sim, Wp)
    A_node = (C * ib[:, :, None]).transpose(0, 2, 1) * A_SCALE  # [g, k, d]
    A_edge = (We[None, :, :] * ib[:, :, None]).transpose(0, 2, 1) * A_SCALE

    def mats_blob(A, np_dt, a_):
        # A: [B, 256 k, 256 d] -> [dev, p, g, dc, kc, m], graph-assigned
        Ad = A[a_.reshape(-1)].astype(np_dt).reshape(
            N_CORES, G, 2, 128, 2, 128)
        return np.ascontiguousarray(Ad.transpose(0, 3, 1, 4, 2, 5))

    mats_n = mats_blob(A_node, BF16, asg_n)
    mats_e = mats_blob(A_edge, edge_np_dt, asg)

    # ---- w tables: wtab[k, ((c*2+typ)*2+kc)*32+m] = w_typ[kc*128+k]*(m==c)
    wt = np.stack([wn, wr]).astype(np.float32)                  # [2, 256]
    eye = np.eye(32, dtype=np.float32)
    wtab = np.einsum("tk,cm->kctm", wt.reshape(2, 2, 128).reshape(4, 128), eye)
    wtab = np.ascontiguousarray(wtab.reshape(128, 32, 2, 2, 32)
                                ).reshape(128, 4 * 32 * 32).astype(BF16)

    # ---- run on 8 cores ----
    nc = _build_bass(regions, n_edge_regions)
    in_maps = [{"items_n": items_n[d], "items_e": items_e[d],
                "mats_n": mats_n[d], "mats_e": mats_e[d], "wtab": wtab}
               for d in range(N_CORES)]
    res = run_bass_kernel_spmd(nc, in_maps, core_ids=list(range(N_CORES)))
    s_rows = np.stack([r["s_out"] for r in res.results])        # [8, 128, 1024]

    # ---- unshard + finish on host ----
    # region R's 512 sums: rows (R%64) + (64 + R%64), cols (R//64)*512
    sum_wn = float(wt[0].astype(BF16).astype(np.float32).sum())
    sum_wr = float(wt[1].astype(BF16).astype(np.float32).sum())
    s_flat = np.empty((N_CORES, n_regions * TILE), np.float32)
    for dev in range(N_CORES):
        S = s_rows[dev]
        for cc in range((n_regions + 63) // 64):
            nt = min(64, n_regions - cc * 64)
            blk = (S[0:nt, cc * TILE:(cc + 1) * TILE]
                   + S[64:64 + nt, cc * TILE:(cc + 1) * TILE])
            s_flat[dev, cc * 64 * TILE:(cc * 64 + nt) * TILE] = blk.reshape(-1)

    def sched_cols(typ, a, n):
        # type-block col range [a, a+n) -> schedule-order s_flat columns
        # (never crosses a region boundary: slots are region-aligned or
        # split below at region granularity)
        c = np.arange(a, a + n)
        base = a - (a % TILE) if typ == 1 else a - nb0 - ((a - nb0) % TILE)
        ri = (c - (nb0 if typ == 0 else 0)) // TILE
        return np.array(Rpos[typ])[ri] * TILE + c % TILE

    state_logits = np.empty(N, np.float32)
    s_e = np.empty(ei.shape[1], np.float32)
    for (typ, gl, j, a, w) in slots:
        for dev in range(N_CORES):
            g = (asg if typ == 1 else asg_n)[dev, gl]
            if typ == 1:
                c = int(np.clip(ce[g] - j * TILE, 0, TILE))
                if c:
                    s_e[estart[g] + j * TILE: estart[g] + j * TILE + c] = \
                        s_flat[dev, sched_cols(1, a, c)] - sum_wr
            else:
                state_logits[nstart[g]:nstart[g + 1]] = \
                    s_flat[dev, sched_cols(0, a, int(cn[g]))] - sum_wn

    rel_logits = np.bincount(dst[eperm], weights=dist[src[eperm]] * s_e,
                             minlength=N).astype(np.float32)

    def seg_softmax(x):
        mx = np.maximum.reduceat(x, nstart[:-1])
        ex = np.exp(x - mx[ni])
        sm = np.add.reduceat(ex, nstart[:-1])
        return ex / sm[ni]

    r = rsim[ni]
    out = r * seg_softmax(rel_logits) + (1.0 - r) * seg_softmax(state_logits)
    return out.astype(np.float32)
